# revision 20
# baseline (speedup 1.0000x reference)
"""Trainium2 Bass kernel for packed-sequence attention (nn_Attention).

Sharding (8 cores): core c handles sequence c//2 and head-group c%2
(8 of 16 heads).  Each core runs an identical SPMD program:
  A) QKV projection (x.T tiles @ wqkv.T column slices, bf16, f32 PSUM)
  B) fused RMSNorm via gpsimd partition_all_reduce (no ones-matmuls, no
     DRAM broadcast roundtrip) + RoPE (pair-swap via SBUF-SBUF DMA in a
     deinterleaved d-basis baked into the host-permuted wqkv rows)
  C) attention per head-pair: S panels for both heads land in one
     [128,1024] PSUM tile -> single batched exp on ACT -> PV matmuls;
     softmax denominators via a bf16 pairwise DVE tree + one gpsimd
     partition_all_reduce per q-panel (removes all row-sum matmuls)
  D) wo matmul over this core's 8 head-chunks -> partial [L, 2048]
Projection and attention of consecutive head-pairs are interleaved at
sub-block granularity so the Tensor engine never waits on the ACT exp
stream.  Host: pairs of cores holding the same sequence have
complementary head groups; their partials are summed (row-parallel TP).
"""

import math
import numpy as np
import ml_dtypes
from contextlib import ExitStack

P = 128
HD = 128
BF = ml_dtypes.bfloat16


def _build_program(L, C, NP, DOUT, n_cores):
    """Build the SPMD per-core program.

    L: tokens per core (sequence length), C: model/contraction dim,
    NP: local head pairs (local heads = 2*NP), DOUT: wo output dim.
    """
    import concourse.bass as bass
    import concourse.mybir as mybir
    import concourse.tile as tile
    from concourse import bacc, bass_isa

    dt = mybir.dt
    AF = mybir.ActivationFunctionType
    OP = mybir.AluOpType
    RED = bass_isa.ReduceOp

    NHL = 2 * NP
    TP = L // 512          # t/q panels
    KC = L // P            # key chunks
    CCH = C // P           # contraction chunks
    JP = DOUT // 512       # output column panels
    scale = 1.0 / math.sqrt(HD)
    EPS = 1e-5

    nc = bacc.Bacc("TRN2", target_bir_lowering=False, debug=False,
                   num_devices=n_cores)

    xT_d = nc.dram_tensor("xT", [C, L], dt.bfloat16, kind="ExternalInput").ap()
    wT_d = nc.dram_tensor("wT", [C, NP * 768], dt.bfloat16, kind="ExternalInput").ap()
    woT_d = nc.dram_tensor("woT", [NHL * HD, DOUT], dt.bfloat16, kind="ExternalInput").ap()
    cosT_d = nc.dram_tensor("cosT", [P, L], dt.bfloat16, kind="ExternalInput").ap()
    sinT_d = nc.dram_tensor("sinT", [P, L], dt.bfloat16, kind="ExternalInput").ap()
    qnw_d = nc.dram_tensor("qnw", [P, 1], dt.float32, kind="ExternalInput").ap()
    knw_d = nc.dram_tensor("knw", [P, 1], dt.float32, kind="ExternalInput").ap()
    out_d = nc.dram_tensor("out", [L, DOUT], dt.float32, kind="ExternalOutput").ap()

    with tile.TileContext(nc) as tc:
        with ExitStack() as ctx:
            const = ctx.enter_context(tc.tile_pool(name="const", bufs=1))
            ps = ctx.enter_context(tc.tile_pool(name="ps", bufs=2, space="PSUM"))
            stream = ctx.enter_context(tc.tile_pool(name="stream", bufs=2))
            qkv = ctx.enter_context(tc.tile_pool(name="qkv", bufs=4))
            opool = ctx.enter_context(tc.tile_pool(name="opool", bufs=NHL))
            work = ctx.enter_context(tc.tile_pool(name="work", bufs=3))
            epool = ctx.enter_context(tc.tile_pool(name="ep", bufs=3))
            dpool = ctx.enter_context(tc.tile_pool(name="dp", bufs=4, space="DRAM"))

            cos_sb = const.tile([P, L], dt.bfloat16, tag="cos", bufs=1)
            sin_sb = const.tile([P, L], dt.bfloat16, tag="sin", bufs=1)
            qnw_sb = const.tile([P, 1], dt.float32, tag="qnw", bufs=1)
            nc.sync.dma_start(qnw_sb[:], qnw_d[:])
            knw_sb = const.tile([P, 1], dt.float32, tag="knw", bufs=1)
            nc.sync.dma_start(knw_sb[:], knw_d[:])
            eps_sb = const.tile([P, 1], dt.float32, tag="eps", bufs=1)
            nc.vector.memset(eps_sb[:], EPS)
            ones_sb = const.tile([P, 1], dt.bfloat16, tag="ones", bufs=1)
            nc.vector.memset(ones_sb[:], 1.0)

            o_tiles = []
            pair_state = {}

            # ---------------- projection blocks for one pair ----------------
            def make_v_blocks(p):
                """v projection: 8 blocks (~3.4us PE each), no ACT usage.
                These interleave into the ACT-bound attention phase."""
                vv = qkv.tile([P, KC, 256], dt.bfloat16, tag="vv", bufs=2,
                              name=f"vv{p}")
                pair_state.setdefault(p, {})["vv"] = vv
                state = {}

                def load_xtc(tp):
                    def run():
                        xtc = stream.tile([P, CCH, 512], dt.bfloat16, tag="xtc",
                                          bufs=2, name=f"xtv{p}_{tp}")
                        xsrc = xT_d[:, bass.ts(tp, 512)].rearrange(
                            "(cc q) w -> q cc w", q=P)
                        for sp in range(8):
                            c0, c1 = sp * CCH // 8, (sp + 1) * CCH // 8
                            nc.sync.dma_start(xtc[:, c0:c1, :], xsrc[:, c0:c1, :])
                        state[f"xtc{tp}"] = xtc
                    return run

                def load_wv():
                    def run():
                        wv = stream.tile([P, CCH, 256], dt.bfloat16, tag="wv",
                                         bufs=1, name=f"wv{p}")
                        wvsrc = wT_d[:, p * 768 + 512:p * 768 + 768].rearrange(
                            "(cc q) w -> q cc w", q=P)
                        for sp in range(4):
                            c0, c1 = sp * CCH // 4, (sp + 1) * CCH // 4
                            nc.sync.dma_start(wv[:, c0:c1, :], wvsrc[:, c0:c1, :])
                        state["wv"] = wv
                    return run

                def v_block(tp, ts_):
                    def run():
                        xtc = state[f"xtc{tp}"]
                        wv = state["wv"]
                        pv = ps.tile([P, 256], dt.float32, tag="acc", bufs=2,
                                     name=f"pv{p}_{tp}_{ts_}")
                        for cc in range(CCH):
                            nc.tensor.matmul(
                                pv[:], xtc[:, cc, ts_ * 128:(ts_ + 1) * 128],
                                wv[:, cc, :],
                                start=(cc == 0), stop=(cc == CCH - 1))
                        nc.vector.tensor_copy(vv[:, tp * 4 + ts_, :], pv[:])
                    return run

                pre = [load_wv(), load_xtc(0)]
                blocks = []
                for tp in range(TP):
                    for ts_ in range(4):
                        b = [v_block(tp, ts_)]
                        if ts_ == 0 and tp + 1 < TP:
                            b.append(load_xtc(tp + 1))
                        blocks.append(b)
                return pre, blocks

            def make_qk_blocks(p):
                """q/k projection + rmsnorm/rope: 16 unit blocks. Sqrt stays
                grouped here so ACT table loads only happen at phase edges."""
                qr_t = [qkv.tile([P, L], dt.bfloat16, tag="qr", bufs=4,
                                 name=f"qr{p}_{i}") for i in range(2)]
                kr_t = [qkv.tile([P, L], dt.bfloat16, tag="kr", bufs=4,
                                 name=f"kr{p}_{i}") for i in range(2)]
                ps_p = pair_state.setdefault(p, {})
                ps_p["qr"] = qr_t
                ps_p["kr"] = kr_t
                state = {}

                def load_xtc(tp):
                    def run():
                        xtc = stream.tile([P, CCH, 512], dt.bfloat16, tag="xtc",
                                          bufs=2, name=f"xtq{p}_{tp}")
                        xsrc = xT_d[:, bass.ts(tp, 512)].rearrange(
                            "(cc q) w -> q cc w", q=P)
                        for sp in range(8):
                            c0, c1 = sp * CCH // 8, (sp + 1) * CCH // 8
                            nc.sync.dma_start(xtc[:, c0:c1, :], xsrc[:, c0:c1, :])
                        state[f"xtc{tp}"] = xtc
                    return run

                def load_wu(u):
                    def run():
                        wu = stream.tile([P, CCH, 128], dt.bfloat16, tag="wu",
                                         bufs=4, name=f"wu{p}_{u}")
                        wsrc = wT_d[:, p * 768 + u * 128:
                                    p * 768 + (u + 1) * 128].rearrange(
                            "(cc q) w -> q cc w", q=P)
                        for sp in range(2):
                            c0, c1 = sp * CCH // 2, (sp + 1) * CCH // 2
                            nc.sync.dma_start(wu[:, c0:c1, :], wsrc[:, c0:c1, :])
                        state[f"wu{u}"] = wu
                    return run

                def unit_block(tp, u):
                    # one q/k unit: 16 matmuls + rmsnorm/rope chain
                    def run():
                        tsl = bass.ts(tp, 512)
                        xtc = state[f"xtc{tp}"]
                        wu = state[f"wu{u}"]
                        pq = ps.tile([P, 512], dt.float32, tag="acc", bufs=2,
                                     name=f"pq{p}_{tp}_{u}")
                        for cc in range(CCH):
                            nc.tensor.matmul(pq[:], wu[:, cc, :], xtc[:, cc, :],
                                             start=(cc == 0), stop=(cc == CCH - 1))
                        wnorm = qnw_sb if u < 2 else knw_sb
                        dest = qr_t[u % 2] if u < 2 else kr_t[u % 2]
                        # psum -> sbuf bf16 copy on ACT (frees the psum bank)
                        qs0 = work.tile([P, 512], dt.bfloat16, tag="qs0", bufs=2)
                        nc.scalar.copy(qs0[:], pq[:])
                        # sum over head-dim (partitions): square + all-reduce
                        q2 = work.tile([P, 512], dt.bfloat16, tag="q2", bufs=2)
                        nc.vector.tensor_mul(q2[:], qs0[:], qs0[:])
                        ssqb = work.tile([P, 512], dt.float32, tag="ssqb", bufs=2)
                        nc.gpsimd.partition_all_reduce(ssqb[:], q2[:],
                                                       channels=P,
                                                       reduce_op=RED.add)
                        rms = work.tile([P, 512], dt.float32, tag="rms", bufs=1)
                        nc.scalar.activation(rms[:], ssqb[:], AF.Sqrt,
                                             bias=eps_sb[:], scale=1.0 / HD)
                        rinv = work.tile([P, 512], dt.float32, tag="rinvp", bufs=1)
                        nc.vector.reciprocal_approx_fast(rinv[:], rms[:])
                        qs = work.tile([P, 512], dt.bfloat16, tag="qs", bufs=3)
                        nc.vector.scalar_tensor_tensor(
                            qs[:], qs0[:], wnorm[:], rinv[:],
                            op0=OP.mult, op1=OP.mult)
                        qsw = work.tile([P, 512], dt.bfloat16, tag="qsw", bufs=2)
                        nc.sync.dma_start(qsw[0:64, :], qs[64:128, :])
                        nc.sync.dma_start(qsw[64:128, :], qs[0:64, :])
                        t1 = work.tile([P, 512], dt.bfloat16, tag="t1", bufs=2)
                        nc.vector.tensor_mul(t1[:], qs[:], cos_sb[:, tsl])
                        t2 = work.tile([P, 512], dt.bfloat16, tag="t2", bufs=2)
                        nc.vector.tensor_mul(t2[:], qsw[:], sin_sb[:, tsl])
                        nc.vector.tensor_add(dest[:, tsl], t1[:], t2[:])
                    return run

                # k-units first within each tp (attention needs full kr before
                # the last q panels), q-units after
                pre = [load_wu(2), load_wu(3), load_xtc(0)]
                blocks = []
                for tp in range(TP):
                    for idx, u in enumerate((2, 3, 0, 1)):
                        b = [unit_block(tp, u)]
                        if idx == 0 and tp + 1 < TP:
                            b.append(load_xtc(tp + 1))
                        if tp == 0 and idx < 2:
                            b.append(load_wu(idx))   # wu0, wu1
                        blocks.append(b)
                return pre, blocks

            # ---------------- attention blocks for one pair ----------------
            def make_attn_blocks(p):
                stp = pair_state.pop(p)
                qr_t, kr_t, vv = stp["qr"], stp["kr"], stp["vv"]
                o_pair = [opool.tile([P, L], dt.bfloat16, tag="o", bufs=NHL,
                                     name=f"o{p}_{i}") for i in range(2)]
                o_tiles.extend(o_pair)
                blocks = []
                for qp in range(TP):
                    qsl = bass.ts(qp, 512)
                    st = {}

                    def qp_start(qp=qp, st=st):
                        def run():
                            st["po"] = [ps.tile([P, 512], dt.float32, tag="po",
                                                bufs=2, name=f"po{p}_{qp}_{l}")
                                        for l in range(2)]
                        return run

                    def s_group(kc0, qp=qp, qsl=qsl, st=st):
                        # S matmuls + batched exp for kc0, kc0+1
                        def run():
                            for kc in (kc0, kc0 + 1):
                                sb = ps.tile([P, 1024], dt.float32, tag="sb",
                                             bufs=2, name=f"sb{p}_{qp}_{kc}")
                                ksl = bass.ts(kc, P)
                                nc.tensor.matmul(sb[:, 0:512], kr_t[0][:, ksl],
                                                 qr_t[0][:, qsl],
                                                 start=True, stop=True)
                                nc.tensor.matmul(sb[:, 512:1024], kr_t[1][:, ksl],
                                                 qr_t[1][:, qsl],
                                                 start=True, stop=True)
                                es = epool.tile([P, 1024], dt.bfloat16, tag="es",
                                                bufs=4, name=f"es{p}_{qp}_{kc}")
                                nc.scalar.activation(es[:], sb[:], AF.Exp,
                                                     scale=scale)
                                st[f"es{kc}"] = es
                        return run

                    def pv_group(kc0, qp=qp, st=st):
                        # PV matmuls for kc0,kc0+1 + softmax-denominator tree
                        def run():
                            po = st["po"]
                            for kc in (kc0, kc0 + 1):
                                es = st[f"es{kc}"]
                                for l in range(2):
                                    nc.tensor.matmul(
                                        po[l][:], vv[:, kc, l * 128:(l + 1) * 128],
                                        es[:, l * 512:(l + 1) * 512],
                                        start=(kc == 0), stop=(kc == KC - 1))
                            # tree level 1
                            j = kc0 // 2
                            s2 = epool.tile([P, 1024], dt.bfloat16, tag="tree",
                                            bufs=5, name=f"s2_{p}_{qp}_{j}")
                            nc.vector.tensor_add(s2[:], st[f"es{kc0}"][:],
                                                 st[f"es{kc0 + 1}"][:])
                            st[f"s2_{j}"] = s2
                            # higher tree levels as pairs become ready
                            lev, idx = 2, j
                            while idx % 2 == 1:
                                a = st[f"s{lev}_{idx - 1}"]
                                b = st[f"s{lev}_{idx}"]
                                nxt = epool.tile([P, 1024], dt.bfloat16,
                                                 tag="tree", bufs=5,
                                                 name=f"s{lev*2}_{p}_{qp}_{idx//2}")
                                nc.vector.tensor_add(nxt[:], a[:], b[:])
                                st[f"s{lev * 2}_{idx // 2}"] = nxt
                                lev, idx = lev * 2, idx // 2
                                if lev == KC:
                                    st["essum"] = nxt
                        return run

                    def qp_finish(qp=qp, qsl=qsl, st=st):
                        def run():
                            po = st["po"]
                            # free po banks immediately (unnormalized copy)
                            ou = [work.tile([P, 512], dt.bfloat16, tag="ou",
                                            bufs=4, name=f"ou{p}_{qp}_{l}")
                                  for l in range(2)]
                            for l in range(2):
                                nc.vector.tensor_copy(ou[l][:], po[l][:])
                            # denominators: all-reduce over keys (partitions)
                            allr = epool.tile([P, 1024], dt.float32, tag="allr",
                                              bufs=1, name=f"allr{p}_{qp}")
                            nc.gpsimd.partition_all_reduce(
                                allr[:], st["essum"][:], channels=P,
                                reduce_op=RED.add)
                            rinvb = epool.tile([P, 1024], dt.float32, tag="rf",
                                               bufs=1, name=f"rf{p}_{qp}")
                            nc.vector.reciprocal_approx_fast(rinvb[:], allr[:])
                            for l in range(2):
                                nc.vector.tensor_mul(
                                    o_pair[l][:, qsl], ou[l][:],
                                    rinvb[:, l * 512:(l + 1) * 512])
                        return run

                    # software-pipelined kc groups: S(g) then PV(g-1)
                    grp = [qp_start(), s_group(0)]
                    for g in range(1, KC // 2):
                        blocks.append(grp)
                        grp = [s_group(2 * g), pv_group(2 * (g - 1))]
                    blocks.append(grp)
                    blocks.append([pv_group(KC - 2), qp_finish()])
                return blocks

            # ---------------- interleaved emission ----------------
            def emit_blocks(blocks):
                for b in blocks:
                    for f in b:
                        f()

            def emit_attn_with_v(ab, vb, skip_first):
                # ab: 36 attn blocks (9 per q-panel), vb: 16 v blocks.
                # Insert one v block after every 2 attn blocks (4 per panel)
                # to fill the ACT-bound exp gaps with v matmuls.
                vi = 0
                first = True
                for qp in range(TP):
                    qb = ab[9 * qp:9 * qp + 9]
                    for j in range(0, 9):
                        if not (first and skip_first):
                            emit_blocks([qb[j]])
                        first = False
                        if j % 2 == 1 and vi < len(vb):
                            emit_blocks([vb[vi]]); vi += 1
                while vi < len(vb):
                    emit_blocks([vb[vi]]); vi += 1

            pending = None    # attn blocks whose first block was pre-emitted
            for p in range(NP):
                vpre, vb = make_v_blocks(p)
                emit_blocks([vpre])
                if p == 0:
                    nc.sync.dma_start(cos_sb[:], cosT_d[:])
                    nc.sync.dma_start(sin_sb[:], sinT_d[:])
                    for b in vb[:8]:
                        emit_blocks([b])
                    vrest = vb[8:]
                else:
                    emit_attn_with_v(pending, vb, skip_first=True)
                    vrest = []
                qpre, qb = make_qk_blocks(p)
                emit_blocks([qpre])
                emit_blocks(qb[:-2])
                for b in vrest:
                    emit_blocks([b])
                # warm up the next attention phase: S matmuls + first exp run
                # during the qk tail, hiding the exp table reload + latency
                pending = make_attn_blocks(p)
                emit_blocks([pending[0]])
                emit_blocks(qb[-2:])
            emit_attn_with_v(pending, [], skip_first=True)

            # ---------------- wo ----------------
            for jp in range(JP):
                jsl = bass.ts(jp, 512)
                wo_jp = stream.tile([P, NHL, 512], dt.bfloat16, tag="wo", bufs=1)
                wosrc = woT_d[:, jsl].rearrange("(h q) j -> q h j", q=P)
                for sp in range(4):
                    h0, h1 = sp * NHL // 4, (sp + 1) * NHL // 4
                    nc.sync.dma_start(wo_jp[:, h0:h1, :], wosrc[:, h0:h1, :])
                for tt in range(L // P):
                    pw = ps.tile([P, 512], dt.float32, tag="acc", bufs=2)
                    for h in range(NHL):
                        nc.tensor.matmul(
                            pw[:], o_tiles[h][:, tt * P:(tt + 1) * P],
                            wo_jp[:, h, :], start=(h == 0), stop=(h == NHL - 1))
                    osb = work.tile([P, 512], dt.float32, tag="outsb", bufs=2)
                    nc.vector.tensor_copy(osb[:], pw[:])
                    nc.sync.dma_start(out_d[tt * P:(tt + 1) * P, jsl], osb[:])

    nc.compile()
    return nc


def _host_prepare(x, rope_cos, rope_sin, wqkv, wo, q_norm_w, k_norm_w,
                  L, C, NP, DOUT, n_cores):
    """Build per-core input dicts."""
    NH_TOT = wqkv.shape[0] // 3 // HD
    NHL = 2 * NP
    perm = np.concatenate([np.arange(0, HD, 2), np.arange(1, HD, 2)])  # deinterleave

    qn_p = np.ascontiguousarray(q_norm_w[perm].reshape(HD, 1)).astype(np.float32)
    kn_p = np.ascontiguousarray(k_norm_w[perm].reshape(HD, 1)).astype(np.float32)

    wq = wqkv[0 * NH_TOT * HD:1 * NH_TOT * HD].reshape(NH_TOT, HD, C)
    wk = wqkv[1 * NH_TOT * HD:2 * NH_TOT * HD].reshape(NH_TOT, HD, C)
    wv = wqkv[2 * NH_TOT * HD:3 * NH_TOT * HD].reshape(NH_TOT, HD, C)

    in_maps = []
    for c in range(n_cores):
        b = c // 2
        hg = c % 2
        heads = list(range(hg * NHL, hg * NHL + NHL))
        xb = x[b * L:(b + 1) * L]                      # [L, C]
        xT = np.ascontiguousarray(xb.T).astype(BF)      # [C, L]

        blocks = []
        for pidx in range(NP):
            h0, h1 = heads[2 * pidx], heads[2 * pidx + 1]
            blocks += [wq[h0][perm], wq[h1][perm],
                       wk[h0][perm], wk[h1][perm],
                       wv[h0], wv[h1]]
        wT = np.ascontiguousarray(np.concatenate(blocks, axis=0).T).astype(BF)

        woT_rows = wo[:, heads[0] * HD:(heads[-1] + 1) * HD].T  # [NHL*HD, DOUT]
        woT = np.ascontiguousarray(woT_rows).astype(BF)

        cosb = rope_cos[b * L:(b + 1) * L].T            # [64, L]
        sinb = rope_sin[b * L:(b + 1) * L].T
        cosT = np.ascontiguousarray(np.concatenate([cosb, cosb], 0)).astype(BF)
        sinT = np.ascontiguousarray(np.concatenate([-sinb, sinb], 0)).astype(BF)

        in_maps.append({
            "xT": xT, "wT": wT, "woT": woT, "cosT": cosT, "sinT": sinT,
            "qnw": qn_p, "knw": kn_p,
        })
    return in_maps


def _reference_numpy(x, rope_cos, rope_sin, cu, max_length,
                     wqkv, wo, q_norm_w, k_norm_w):
    """Pure-numpy fallback (exact reference math) for non-uniform cu."""
    T, dim = x.shape
    nh = dim // HD
    qkv = (x @ wqkv.T).reshape(T, 3, nh, HD)
    q, k, v = qkv[:, 0], qkv[:, 1], qkv[:, 2]

    def rmsnorm(t, w):
        return t / np.sqrt((t * t).mean(-1, keepdims=True) + 1e-5) * w

    def rope(t):
        tr = t.reshape(t.shape[:-1] + (HD // 2, 2))
        e, o = tr[..., 0], tr[..., 1]
        cc = rope_cos[:, None, :]
        ss = rope_sin[:, None, :]
        return np.stack([e * cc - o * ss, e * ss + o * cc], -1).reshape(t.shape)

    q = rope(rmsnorm(q, q_norm_w))
    k = rope(rmsnorm(k, k_norm_w))
    o = np.zeros((T, nh, HD), np.float32)
    nb = len(cu) - 1
    for i in range(nb):
        s, e_ = int(cu[i]), int(cu[i + 1])
        if e_ <= s:
            continue
        qs_, ks_, vs_ = q[s:e_], k[s:e_], v[s:e_]
        sc = np.einsum("lhd,mhd->hlm", qs_, ks_) / math.sqrt(HD)
        sc = sc - sc.max(-1, keepdims=True)
        a = np.exp(sc)
        a /= a.sum(-1, keepdims=True)
        o[s:e_] = np.einsum("hlm,mhd->lhd", a, vs_)
    return (o.reshape(T, dim) @ wo.T).astype(np.float32)


def kernel(x, rope_cos, rope_sin, cu, max_length, wqkv, wo, q_norm_w, k_norm_w):
    x = np.asarray(x, np.float32)
    rope_cos = np.asarray(rope_cos, np.float32)
    rope_sin = np.asarray(rope_sin, np.float32)
    cu = np.asarray(cu)
    wqkv = np.asarray(wqkv, np.float32)
    wo = np.asarray(wo, np.float32)
    q_norm_w = np.asarray(q_norm_w, np.float32)
    k_norm_w = np.asarray(k_norm_w, np.float32)

    T, C = x.shape
    N_CORES = 8
    L = T // 4
    expect_cu = np.arange(5) * L
    if (len(cu) != 5 or not np.array_equal(np.asarray(cu).ravel(), expect_cu)
            or T % 4 != 0 or L % 512 != 0 or C % P != 0):
        return _reference_numpy(x, rope_cos, rope_sin, cu, max_length,
                                wqkv, wo, q_norm_w, k_norm_w)

    NP = (C // HD) // 2 // 2          # local head pairs = NH/2/2
    DOUT = wo.shape[0]

    from concourse.bass_utils import run_bass_kernel_spmd

    nc = _build_program(L, C, NP, DOUT, N_CORES)
    in_maps = _host_prepare(x, rope_cos, rope_sin, wqkv, wo, q_norm_w, k_norm_w,
                            L, C, NP, DOUT, N_CORES)
    res = run_bass_kernel_spmd(nc, in_maps, list(range(N_CORES)))

    out = np.empty((T, DOUT), np.float32)
    for b in range(4):
        out[b * L:(b + 1) * L] = (res.results[2 * b]["out"]
                                  + res.results[2 * b + 1]["out"])
    return out
